# revision 1
# baseline (speedup 1.0000x reference)
"""Causal attention (B=4, S=2048, D=1024) on 8 Trainium2 NeuronCores.

Sharding: 2 cores per batch element. Within a batch, the 8 query blocks of
256 rows are split between the two cores by parity (fold 0 takes odd blocks,
fold 1 takes even blocks) so causal-attention work is balanced. Each core
computes Q for its own 1024 query rows, and K/V for the full 2048 context
rows (duplicated across the pair — cheaper than a collective here).

All matmuls run in bf16 (fp32 accumulate in PSUM) with N=512 moving operands:
the PE streams one 128x128x512 matmul every ~215 ns with the bf16
fast-weight-load fully hidden, and bf16 halves all DMA traffic and SBUF
footprints (K^T, V, Q^T, and exp(S) all stay resident / tiny). End-to-end
absmax-relative error vs the fp32 reference is ~3e-3.

Layout trick: scores are computed transposed (k on partitions, q on free dim)
via S^T = K^T.T @ Q^T, so no transpose of the softmax matrix is needed:
exp(S^T) tiles feed attn@V directly as the stationary operand, producing the
output in natural [q, o] layout. Scores for two adjacent 256-row query slots
are computed together (N=512) over the union of their causal depths; the 0/1
causal masks (streamed per-core from the host, so one SPMD program serves
both folds) zero both the diagonal parts and the over-computed region, which
also keeps the softmax denominators correct. Softmax skips max-subtraction
(scores/sqrt(d) are ~N(0,1) here; exp cannot overflow), with denominators
from a ones-column matmul per 128-query chunk.
"""

import sys

sys.path.insert(0, "/opt/trn_rl_repo")

import ml_dtypes
import numpy as np

import concourse.bass as bass  # noqa: F401
import concourse.mybir as mybir
import concourse.tile as tile
from concourse import bacc
from concourse.bass_utils import run_bass_kernel_spmd

F32 = mybir.dt.float32
BF16 = mybir.dt.bfloat16
AF = mybir.ActivationFunctionType

B, S, D = 4, 2048, 1024
P = 128
DC = D // P  # 8 contraction chunks
OC = D // P  # 8 output-feature chunks
TC = S // P  # 16 context chunks
N_CORES = 8
SLOTS = 4  # query slots of 256 rows per core
QB = 256
# Padded causal depth (in 128-wide k tiles) per slot, fold-uniform:
# fold 0 owns global 256-blocks [1,3,5,7] (true depths 4,8,12,16),
# fold 1 owns [0,2,4,6] (true depths 2,6,10,14) -> padded to fold-0 depths.
KT_COUNTS = [4, 8, 12, 16]
FOLD_QBLOCKS = {0: [1, 3, 5, 7], 1: [0, 2, 4, 6]}
# Slot pairs (0,1) and (2,3) share one N=512 scores pass over the union depth.
PAIR_DEPTH = [KT_COUNTS[1], KT_COUNTS[3]]  # [8, 16]
N_MASK = sum(PAIR_DEPTH) - 8  # pair0: kt 0..7 masked; pair1: kt 8..15 masked
SCALE = 1.0 / np.sqrt(np.float32(D))


def _build_nc(repeat: int = 1):
    nc = bacc.Bacc("TRN2", target_bir_lowering=False, debug=False, num_devices=N_CORES)

    xT_d = nc.declare_dram_parameter("xT", [D, S], BF16, isOutput=False)
    xTq_d = nc.declare_dram_parameter("xTq", [D, SLOTS * QB], BF16, isOutput=False)
    wq_d = nc.declare_dram_parameter("wqT", [D, D], BF16, isOutput=False)
    wk_d = nc.declare_dram_parameter("wkT", [D, D], BF16, isOutput=False)
    wv_d = nc.declare_dram_parameter("wvT", [D, D], BF16, isOutput=False)
    mask_d = nc.declare_dram_parameter(
        "masks", [N_MASK, P, 2 * QB], BF16, isOutput=False
    )
    out_d = nc.declare_dram_parameter("out", [SLOTS * QB, D], F32, isOutput=True)

    xT = xT_d[:].rearrange("(dc p) t -> p dc t", p=P)  # [128, 8, 2048]
    xTq = xTq_d[:].rearrange("(dc p) q -> p dc q", p=P)  # [128, 8, 1024]
    wq = wq_d[:].rearrange("(dc p) o -> p dc o", p=P)
    wk = wk_d[:].rearrange("(dc p) o -> p dc o", p=P)
    wv = wv_d[:].rearrange("(dc p) o -> p dc o", p=P)
    out_r = out_d[:].rearrange("(qc p) o -> p qc o", p=P)  # [128, 8, 1024]

    with tile.TileContext(nc, pool_alloc_mode="queue") as tc:
      for _rep in range(repeat):
        with tc.tile_pool(name="resident", bufs=1) as res_pool:
            kt_res = res_pool.tile([P, OC, S], BF16, name="kt_res")
            v_res = res_pool.tile([P, TC, D], BF16, name="v_res")
            qt_res = res_pool.tile([P, OC, SLOTS * QB], BF16, name="qt_res")
            ones2 = res_pool.tile([P, 2], BF16, name="ones2")
            nc.vector.memset(ones2[:], 1.0)

            # ---- Phase Q: Q^T = Wq^T.T @ xTq -> qt_res (SBUF) --------------
            with (
                tc.tile_pool(name="wq_pool", bufs=1) as wpool,
                tc.tile_pool(name="xq_pool", bufs=2) as xpool,
                tc.tile_pool(name="psum_q", bufs=4, space="PSUM") as pspool,
            ):
                w_t = wpool.tile([P, DC, D], BF16, name="wq_t")
                for dc in range(DC):  # chunked so first matmuls start early
                    nc.sync.dma_start(w_t[:, dc, :], wq[:, dc, :])
                for qt in range(2):  # 512-wide query column tiles
                    x_t = xpool.tile([P, DC, 512], BF16, name="xq_t")
                    for dc in range(DC):
                        nc.sync.dma_start(
                            x_t[:, dc, :], xTq[:, dc, 512 * qt : 512 * (qt + 1)]
                        )
                    for oc in range(OC):
                        ps = pspool.tile([P, 512], F32, name="ps_q")
                        for dc in range(DC):
                            nc.tensor.matmul(
                                ps[:],
                                lhsT=w_t[:, dc, P * oc : P * (oc + 1)],
                                rhs=x_t[:, dc, :],
                                start=(dc == 0),
                                stop=(dc == DC - 1),
                            )
                        nc.vector.tensor_copy(
                            qt_res[:, oc, 512 * qt : 512 * (qt + 1)], ps[:]
                        )

            # ---- Phase KV (merged, one pass over xT): K^T and V ------------
            with (
                tc.tile_pool(name="wk_pool", bufs=1) as wkpool,
                tc.tile_pool(name="wv_pool", bufs=1) as wvpool,
                tc.tile_pool(name="xkv_pool", bufs=2) as xpool,
                tc.tile_pool(name="psum_kv", bufs=6, space="PSUM") as pspool,
            ):
                wk_t = wkpool.tile([P, DC, D], BF16, name="wk_t")
                wv_t = wvpool.tile([P, DC, D], BF16, name="wv_t")
                for dc in range(DC):
                    nc.sync.dma_start(wk_t[:, dc, :], wk[:, dc, :])
                    nc.sync.dma_start(wv_t[:, dc, :], wv[:, dc, :])
                for tt in range(4):  # 512-wide context tiles
                    x_t = xpool.tile([P, DC, 512], BF16, name="xkv_t")
                    for dc in range(DC):
                        nc.sync.dma_start(
                            x_t[:, dc, :], xT[:, dc, 512 * tt : 512 * (tt + 1)]
                        )
                    # K^T: [o-part, t]
                    for oc in range(OC):
                        ps = pspool.tile([P, 512], F32, name="ps_k", tag="ps_kv")
                        for dc in range(DC):
                            nc.tensor.matmul(
                                ps[:],
                                lhsT=wk_t[:, dc, P * oc : P * (oc + 1)],
                                rhs=x_t[:, dc, :],
                                start=(dc == 0),
                                stop=(dc == DC - 1),
                            )
                        nc.vector.tensor_copy(
                            kt_res[:, oc, 512 * tt : 512 * (tt + 1)], ps[:]
                        )
                    # V: [t-part, o]
                    for tci in range(4):
                        tcg = 4 * tt + tci
                        for ot in range(2):
                            ps = pspool.tile([P, 512], F32, name="ps_v", tag="ps_kv")
                            for dc in range(DC):
                                nc.tensor.matmul(
                                    ps[:],
                                    lhsT=x_t[:, dc, P * tci : P * (tci + 1)],
                                    rhs=wv_t[:, dc, 512 * ot : 512 * (ot + 1)],
                                    start=(dc == 0),
                                    stop=(dc == DC - 1),
                                )
                            nc.vector.tensor_copy(
                                v_res[:, tcg, 512 * ot : 512 * (ot + 1)], ps[:]
                            )

            # ---- Phase A: attention, one slot-pair (512 q) at a time -------
            with (
                tc.tile_pool(name="es_pool", bufs=16) as epool,
                tc.tile_pool(name="mk_pool", bufs=2) as mpool,
                tc.tile_pool(name="ob_pool", bufs=3) as opool,
                tc.tile_pool(name="rc_pool", bufs=2) as rpool,
                tc.tile_pool(name="psum_s", bufs=2, space="PSUM") as pss,
                tc.tile_pool(name="psum_o", bufs=4, space="PSUM") as pso_pool,
                tc.tile_pool(name="psum_d", bufs=2, space="PSUM") as psd_pool,
            ):
                mask_i = 0
                for p in range(2):  # slot pairs (0,1), (2,3)
                    depth = PAIR_DEPTH[p]
                    # scores + exp + mask over the union depth
                    es_tiles = []
                    for kt in range(depth):
                        ps_s = pss.tile([P, 512], F32, name="ps_s")
                        for oc in range(OC):
                            nc.tensor.matmul(
                                ps_s[:],
                                lhsT=kt_res[:, oc, P * kt : P * (kt + 1)],
                                rhs=qt_res[:, oc, 512 * p : 512 * (p + 1)],
                                start=(oc == 0),
                                stop=(oc == OC - 1),
                            )
                        es = epool.tile([P, 512], BF16, name="es")
                        nc.scalar.activation(es[:], ps_s[:], AF.Exp, scale=SCALE)
                        if p == 1 and kt < 8:
                            pass  # both slots fully valid, no mask needed
                        else:
                            mt = mpool.tile([P, 512], BF16, name="mask_t")
                            nc.sync.dma_start(mt[:], mask_d[mask_i])
                            nc.vector.tensor_mul(out=es[:], in0=es[:], in1=mt[:])
                            mask_i += 1
                        es_tiles.append(es)
                    # attn@V: two sweeps (slot A: qcc 0,1; slot B: qcc 2,3)
                    for sw, qccs in enumerate(((0, 1), (2, 3))):
                        sdepth = KT_COUNTS[2 * p + sw]
                        pso = {
                            (qcc, ot): pso_pool.tile([P, 512], F32, name="ps_o")
                            for qcc in qccs
                            for ot in range(2)
                        }
                        psd = {
                            qcc: psd_pool.tile([P, 2], F32, name="ps_d")
                            for qcc in qccs
                        }
                        for kt in range(sdepth):
                            first, last = (kt == 0), (kt == sdepth - 1)
                            for qcc in qccs:
                                lhs = es_tiles[kt][:, P * qcc : P * (qcc + 1)]
                                for ot in range(2):
                                    nc.tensor.matmul(
                                        pso[(qcc, ot)][:],
                                        lhsT=lhs,
                                        rhs=v_res[:, kt, 512 * ot : 512 * (ot + 1)],
                                        start=first,
                                        stop=last,
                                    )
                                nc.tensor.matmul(
                                    psd[qcc][:],
                                    lhsT=lhs,
                                    rhs=ones2[:],
                                    start=first,
                                    stop=last,
                                )
                        for qcc in qccs:
                            rc = rpool.tile([P, 1], F32, name="rc")
                            nc.vector.reciprocal(rc[:], psd[qcc][:, 0:1])
                            for ot in range(2):
                                ob = opool.tile([P, 512], F32, name="ob")
                                nc.scalar.activation(
                                    ob[:], pso[(qcc, ot)][:], AF.Copy, scale=rc[:]
                                )
                                nc.sync.dma_start(
                                    out_r[:, 4 * p + qcc, 512 * ot : 512 * (ot + 1)],
                                    ob[:],
                                )

    nc.compile()
    if not nc.is_finalized():
        nc.finalize()
    return nc


def _build_masks(fold: int) -> np.ndarray:
    """0/1 masks [N_MASK, 128, 512]; cols 0:256 = slot 2p, 256:512 = slot 2p+1."""
    tiles = []
    ki = np.arange(P)[:, None]
    qi = np.arange(QB)[None, :]
    for p in range(2):
        lo = 8 if p == 1 else 0  # pair1 kt<8 is fully valid for both folds
        for kt in range(lo, PAIR_DEPTH[p]):
            k0 = kt * P
            halves = []
            for s in (2 * p, 2 * p + 1):
                q0 = FOLD_QBLOCKS[fold][s] * QB
                halves.append(((q0 + qi) >= (k0 + ki)).astype(np.float32))
            tiles.append(np.concatenate(halves, axis=1))
    return np.ascontiguousarray(np.stack(tiles).astype(ml_dtypes.bfloat16))


def build_in_maps(inputs):
    x = np.asarray(inputs["inputs"], dtype=np.float32)
    bf = ml_dtypes.bfloat16
    wqT = np.ascontiguousarray(np.asarray(inputs["Wq"], dtype=np.float32).T.astype(bf))
    wkT = np.ascontiguousarray(np.asarray(inputs["Wk"], dtype=np.float32).T.astype(bf))
    wvT = np.ascontiguousarray(np.asarray(inputs["Wv"], dtype=np.float32).T.astype(bf))

    masks = {f: _build_masks(f) for f in (0, 1)}
    in_maps = []
    for c in range(N_CORES):
        b, f = c // 2, c % 2
        xT = np.ascontiguousarray(x[b].T.astype(bf))  # [D, S]
        xTq = np.ascontiguousarray(
            np.concatenate(
                [xT[:, qb * QB : (qb + 1) * QB] for qb in FOLD_QBLOCKS[f]], axis=1
            )
        )
        in_maps.append(
            {
                "xT": xT,
                "xTq": xTq,
                "wqT": wqT,
                "wkT": wkT,
                "wvT": wvT,
                "masks": masks[f],
            }
        )
    return in_maps


def kernel(**inputs: np.ndarray) -> np.ndarray:
    in_maps = build_in_maps(inputs)
    nc = _build_nc()
    res = run_bass_kernel_spmd(nc, in_maps, core_ids=list(range(N_CORES)))

    out = np.empty((B, S, D), dtype=np.float32)
    for c in range(N_CORES):
        b, f = c // 2, c % 2
        o = res.results[c]["out"]  # [1024, 1024] rows in slot order
        for s, qb in enumerate(FOLD_QBLOCKS[f]):
            out[b, qb * QB : (qb + 1) * QB, :] = o[s * QB : (s + 1) * QB, :]
    return out



# revision 3
# speedup vs baseline: 1.0062x; 1.0062x over previous
"""Causal attention (B=4, S=2048, D=1024) on 8 Trainium2 NeuronCores.

Sharding: 2 cores per batch element. Within a batch, the 8 query blocks of
256 rows are split between the two cores by parity (fold 0 takes odd blocks,
fold 1 takes even blocks) so causal-attention work is balanced. Each core
computes Q for its own 1024 query rows. K/V projection is split across the
pair: each core projects the 1024 context columns of its own parity-
interleaved 128-blocks (fold f owns global blocks g with g%2==f), then a
pair-wise AllGather exchanges the halves. The exchange runs in 2 chunks so
chunk 0 (global context [0:1024), exactly what the first slot-pair of the
attention needs) is in flight while the second chunk is still being
projected, and the Q projection overlaps the collective.

All matmuls run in bf16 (fp32 accumulate in PSUM) with N=512 moving operands:
the PE streams one 128x128x512 matmul every ~215 ns with the bf16
fast-weight-load fully hidden, and bf16 halves all DMA traffic and SBUF
footprints (K^T, V, Q^T, and exp(S) all stay resident / tiny).

Layout trick: scores are computed transposed (k on partitions, q on free dim)
via S^T = K^T.T @ Q^T, so no transpose of the softmax matrix is needed:
exp(S^T) tiles feed attn@V directly as the stationary operand, producing the
output in natural [q, o] layout. Scores for two adjacent 256-row query slots
are computed together (N=512) over the union of their causal depths; the 0/1
causal masks (streamed per-core from the host, so one SPMD program serves
both folds) zero both the diagonal parts and the over-computed region, which
also keeps the softmax denominators correct. Softmax skips max-subtraction
(scores/sqrt(d) are ~N(0,1) here; exp cannot overflow), with denominators
from a ones-column matmul per 128-query chunk.
"""

import sys

sys.path.insert(0, "/opt/trn_rl_repo")

import ml_dtypes
import numpy as np

import concourse.bass as bass  # noqa: F401
import concourse.mybir as mybir
import concourse.tile as tile
from concourse import bacc
from concourse.bass_utils import run_bass_kernel_spmd

F32 = mybir.dt.float32
BF16 = mybir.dt.bfloat16
AF = mybir.ActivationFunctionType

B, S, D = 4, 2048, 1024
P = 128
DC = D // P  # 8 contraction chunks
OC = D // P  # 8 output-feature chunks
TC = S // P  # 16 context chunks
N_CORES = 8
SLOTS = 4  # query slots of 256 rows per core
QB = 256
# Padded causal depth (in 128-wide k tiles) per slot, fold-uniform:
# fold 0 owns global 256-blocks [1,3,5,7] (true depths 4,8,12,16),
# fold 1 owns [0,2,4,6] (true depths 2,6,10,14) -> padded to fold-0 depths.
KT_COUNTS = [4, 8, 12, 16]
FOLD_QBLOCKS = {0: [1, 3, 5, 7], 1: [0, 2, 4, 6]}
# Slot pairs (0,1) and (2,3) share one N=512 scores pass over the union depth.
PAIR_DEPTH = [KT_COUNTS[1], KT_COUNTS[3]]  # [8, 16]
N_MASK = sum(PAIR_DEPTH) - 8  # pair0: kt 0..7 masked; pair1: kt 8..15 masked
SCALE = 1.0 / np.sqrt(np.float32(D))
AG_GROUPS = [[0, 1], [2, 3], [4, 5], [6, 7]]
AGW = OC * 512 + 4 * D  # ag payload cols per chunk: K [8,512] + V [4,1024]


def _build_nc(repeat: int = 1):
    nc = bacc.Bacc("TRN2", target_bir_lowering=False, debug=False, num_devices=N_CORES)

    xTc_d = nc.declare_dram_parameter("xTc", [D, S // 2], BF16, isOutput=False)
    xTq_d = nc.declare_dram_parameter("xTq", [D, SLOTS * QB], BF16, isOutput=False)
    wq_d = nc.declare_dram_parameter("wqT", [D, D], BF16, isOutput=False)
    wk_d = nc.declare_dram_parameter("wkT", [D, D], BF16, isOutput=False)
    wv_d = nc.declare_dram_parameter("wvT", [D, D], BF16, isOutput=False)
    mask_d = nc.declare_dram_parameter(
        "masks", [N_MASK, P, 2 * QB], BF16, isOutput=False
    )
    out_d = nc.declare_dram_parameter("out", [SLOTS * QB, D], F32, isOutput=True)

    xTc = xTc_d[:].rearrange("(dc p) t -> p dc t", p=P)  # [128, 8, 1024]
    xTq = xTq_d[:].rearrange("(dc p) q -> p dc q", p=P)  # [128, 8, 1024]
    wq = wq_d[:].rearrange("(dc p) o -> p dc o", p=P)
    wk = wk_d[:].rearrange("(dc p) o -> p dc o", p=P)
    wv = wv_d[:].rearrange("(dc p) o -> p dc o", p=P)
    out_r = out_d[:].rearrange("(qc p) o -> p qc o", p=P)  # [128, 8, 1024]

    with tile.TileContext(nc, pool_alloc_mode="queue") as tc:
      for _rep in range(repeat):
        with tc.tile_pool(name="resident", bufs=1) as res_pool:
            kt_res = res_pool.tile([P, OC, S], BF16, name="kt_res")
            v_res = res_pool.tile([P, TC, D], BF16, name="v_res")
            qt_res = res_pool.tile([P, OC, SLOTS * QB], BF16, name="qt_res")
            ones2 = res_pool.tile([P, 2], BF16, name="ones2")
            nc.vector.memset(ones2[:], 1.0)

            # ---- Phase KV: project OWN parity context half, pair-AllGather -
            with (
                tc.tile_pool(name="wk_pool", bufs=1) as wkpool,
                tc.tile_pool(name="wv_pool", bufs=1) as wvpool,
                tc.tile_pool(name="xkv_pool", bufs=2) as xpool,
                tc.tile_pool(name="kv_stage", bufs=2) as stpool,
                tc.tile_pool(name="ag_dram", bufs=1, space="DRAM") as dpool,
                tc.tile_pool(name="psum_kv", bufs=6, space="PSUM") as pspool,
            ):
                wk_t = wkpool.tile([P, DC, D], BF16, name="wk_t")
                wv_t = wvpool.tile([P, DC, D], BF16, name="wv_t")
                for dc in range(DC):
                    nc.sync.dma_start(wk_t[:, dc, :], wk[:, dc, :])
                    nc.sync.dma_start(wv_t[:, dc, :], wv[:, dc, :])
                for ch in range(2):  # 512 own-ctx cols per chunk = global half
                    x_t = xpool.tile([P, DC, 512], BF16, name="xkv_t")
                    for dc in range(DC):
                        nc.sync.dma_start(
                            x_t[:, dc, :], xTc[:, dc, 512 * ch : 512 * (ch + 1)]
                        )
                    kst = stpool.tile([P, OC * 512], BF16, name="k_stage")
                    vst = stpool.tile([P, 4 * D], BF16, name="v_stage")
                    # K^T own columns: [o-part, own ctx]
                    for oc in range(OC):
                        ps = pspool.tile([P, 512], F32, name="ps_k", tag="ps_kv")
                        for dc in range(DC):
                            nc.tensor.matmul(
                                ps[:],
                                lhsT=wk_t[:, dc, P * oc : P * (oc + 1)],
                                rhs=x_t[:, dc, :],
                                start=(dc == 0),
                                stop=(dc == DC - 1),
                            )
                        nc.vector.tensor_copy(
                            kst[:, 512 * oc : 512 * (oc + 1)], ps[:]
                        )
                    # V own rows: [ctx-part, o]
                    for j in range(4):
                        for ot in range(2):
                            ps = pspool.tile([P, 512], F32, name="ps_v", tag="ps_kv")
                            for dc in range(DC):
                                nc.tensor.matmul(
                                    ps[:],
                                    lhsT=x_t[:, dc, P * j : P * (j + 1)],
                                    rhs=wv_t[:, dc, 512 * ot : 512 * (ot + 1)],
                                    start=(dc == 0),
                                    stop=(dc == DC - 1),
                                )
                            nc.vector.tensor_copy(
                                vst[:, D * j + 512 * ot : D * j + 512 * (ot + 1)],
                                ps[:],
                            )
                    agin = dpool.tile([P, AGW], BF16, name=f"ag_in{ch}")
                    agout = dpool.tile([2, P, AGW], BF16, name=f"ag_out{ch}")
                    nc.sync.dma_start(agin[:, 0 : OC * 512], kst[:])
                    nc.sync.dma_start(agin[:, OC * 512 : AGW], vst[:])
                    nc.gpsimd.collective_compute(
                        "AllGather",
                        mybir.AluOpType.bypass,
                        replica_groups=AG_GROUPS,
                        ins=[agin.opt()],
                        outs=[agout.opt()],
                    )
                    # Unpack both ranks' halves into GLOBAL context order:
                    # rank r's local block j of chunk ch is global block
                    # 8*ch + 2*j + r.
                    ktdst = kt_res[:, :, 1024 * ch : 1024 * (ch + 1)].rearrange(
                        "p oc (j r c) -> p oc j r c", j=4, r=2
                    )
                    vdst = v_res[:, 8 * ch : 8 * (ch + 1), :].rearrange(
                        "p (j r) o -> p j r o", j=4
                    )
                    for r in range(2):
                        agk = agout[r, :, 0 : OC * 512].rearrange(
                            "p (oc j c) -> p oc j c", oc=OC, j=4
                        )
                        agv = agout[r, :, OC * 512 : AGW].rearrange(
                            "p (j o) -> p j o", j=4
                        )
                        for j in range(4):
                            nc.scalar.dma_start(
                                ktdst[:, :, j, r, :], agk[:, :, j, :]
                            )
                        nc.scalar.dma_start(vdst[:, :, r, :], agv[:])

            # ---- Phase Q: Q^T = Wq^T.T @ xTq -> qt_res (SBUF) --------------
            with (
                tc.tile_pool(name="wq_pool", bufs=1) as wpool,
                tc.tile_pool(name="xq_pool", bufs=2) as xqpool,
                tc.tile_pool(name="psum_q", bufs=4, space="PSUM") as pspool,
            ):
                w_t = wpool.tile([P, DC, D], BF16, name="wq_t")
                for dc in range(DC):  # chunked so first matmuls start early
                    nc.sync.dma_start(w_t[:, dc, :], wq[:, dc, :])
                for qt in range(2):  # 512-wide query column tiles
                    x_t = xqpool.tile([P, DC, 512], BF16, name="xq_t")
                    for dc in range(DC):
                        nc.sync.dma_start(
                            x_t[:, dc, :], xTq[:, dc, 512 * qt : 512 * (qt + 1)]
                        )
                    for oc in range(OC):
                        ps = pspool.tile([P, 512], F32, name="ps_q")
                        for dc in range(DC):
                            nc.tensor.matmul(
                                ps[:],
                                lhsT=w_t[:, dc, P * oc : P * (oc + 1)],
                                rhs=x_t[:, dc, :],
                                start=(dc == 0),
                                stop=(dc == DC - 1),
                            )
                        nc.vector.tensor_copy(
                            qt_res[:, oc, 512 * qt : 512 * (qt + 1)], ps[:]
                        )

            # ---- Phase A: attention, one slot-pair (512 q) at a time -------
            with (
                tc.tile_pool(name="es_pool", bufs=16) as epool,
                tc.tile_pool(name="mk_pool", bufs=2) as mpool,
                tc.tile_pool(name="ob_pool", bufs=3) as opool,
                tc.tile_pool(name="rc_pool", bufs=2) as rpool,
                tc.tile_pool(name="psum_s", bufs=2, space="PSUM") as pss,
                tc.tile_pool(name="psum_o", bufs=4, space="PSUM") as pso_pool,
                tc.tile_pool(name="psum_d", bufs=2, space="PSUM") as psd_pool,
            ):
                mask_i = 0
                for p in range(2):  # slot pairs (0,1), (2,3)
                    depth = PAIR_DEPTH[p]
                    # scores + exp + mask over the union depth
                    es_tiles = []
                    for kt in range(depth):
                        ps_s = pss.tile([P, 512], F32, name="ps_s")
                        for oc in range(OC):
                            nc.tensor.matmul(
                                ps_s[:],
                                lhsT=kt_res[:, oc, P * kt : P * (kt + 1)],
                                rhs=qt_res[:, oc, 512 * p : 512 * (p + 1)],
                                start=(oc == 0),
                                stop=(oc == OC - 1),
                            )
                        es = epool.tile([P, 512], BF16, name="es")
                        nc.scalar.activation(es[:], ps_s[:], AF.Exp, scale=SCALE)
                        if p == 1 and kt < 8:
                            pass  # both slots fully valid, no mask needed
                        else:
                            mt = mpool.tile([P, 512], BF16, name="mask_t")
                            nc.sync.dma_start(mt[:], mask_d[mask_i])
                            nc.vector.tensor_mul(out=es[:], in0=es[:], in1=mt[:])
                            mask_i += 1
                        es_tiles.append(es)
                    # attn@V: two sweeps (slot A: qcc 0,1; slot B: qcc 2,3)
                    for sw, qccs in enumerate(((0, 1), (2, 3))):
                        sdepth = KT_COUNTS[2 * p + sw]
                        pso = {
                            (qcc, ot): pso_pool.tile([P, 512], F32, name="ps_o")
                            for qcc in qccs
                            for ot in range(2)
                        }
                        psd = {
                            qcc: psd_pool.tile([P, 2], F32, name="ps_d")
                            for qcc in qccs
                        }
                        for kt in range(sdepth):
                            first, last = (kt == 0), (kt == sdepth - 1)
                            for qcc in qccs:
                                lhs = es_tiles[kt][:, P * qcc : P * (qcc + 1)]
                                for ot in range(2):
                                    nc.tensor.matmul(
                                        pso[(qcc, ot)][:],
                                        lhsT=lhs,
                                        rhs=v_res[:, kt, 512 * ot : 512 * (ot + 1)],
                                        start=first,
                                        stop=last,
                                    )
                                nc.tensor.matmul(
                                    psd[qcc][:],
                                    lhsT=lhs,
                                    rhs=ones2[:],
                                    start=first,
                                    stop=last,
                                )
                        for qcc in qccs:
                            rc = rpool.tile([P, 1], F32, name="rc")
                            nc.vector.reciprocal(rc[:], psd[qcc][:, 0:1])
                            for ot in range(2):
                                ob = opool.tile([P, 512], F32, name="ob")
                                nc.scalar.activation(
                                    ob[:], pso[(qcc, ot)][:], AF.Copy, scale=rc[:]
                                )
                                nc.sync.dma_start(
                                    out_r[:, 4 * p + qcc, 512 * ot : 512 * (ot + 1)],
                                    ob[:],
                                )

    nc.compile()
    if not nc.is_finalized():
        nc.finalize()
    return nc


def _build_masks(fold: int) -> np.ndarray:
    """0/1 masks [N_MASK, 128, 512]; cols 0:256 = slot 2p, 256:512 = slot 2p+1."""
    tiles = []
    ki = np.arange(P)[:, None]
    qi = np.arange(QB)[None, :]
    for p in range(2):
        lo = 8 if p == 1 else 0  # pair1 kt<8 is fully valid for both folds
        for kt in range(lo, PAIR_DEPTH[p]):
            k0 = kt * P
            halves = []
            for s in (2 * p, 2 * p + 1):
                q0 = FOLD_QBLOCKS[fold][s] * QB
                halves.append(((q0 + qi) >= (k0 + ki)).astype(np.float32))
            tiles.append(np.concatenate(halves, axis=1))
    return np.ascontiguousarray(np.stack(tiles).astype(ml_dtypes.bfloat16))


def build_in_maps(inputs):
    x = np.asarray(inputs["inputs"], dtype=np.float32)
    bf = ml_dtypes.bfloat16
    wqT = np.ascontiguousarray(np.asarray(inputs["Wq"], dtype=np.float32).T.astype(bf))
    wkT = np.ascontiguousarray(np.asarray(inputs["Wk"], dtype=np.float32).T.astype(bf))
    wvT = np.ascontiguousarray(np.asarray(inputs["Wv"], dtype=np.float32).T.astype(bf))

    masks = {f: _build_masks(f) for f in (0, 1)}
    in_maps = []
    for c in range(N_CORES):
        b, f = c // 2, c % 2
        xT = np.ascontiguousarray(x[b].T.astype(bf))  # [D, S]
        xTq = np.ascontiguousarray(
            np.concatenate(
                [xT[:, qb * QB : (qb + 1) * QB] for qb in FOLD_QBLOCKS[f]], axis=1
            )
        )
        # Own parity context blocks, packed: local j -> global block 2*j + f.
        xTc = np.ascontiguousarray(
            np.concatenate(
                [xT[:, g * P : (g + 1) * P] for g in range(f, TC, 2)], axis=1
            )
        )
        in_maps.append(
            {
                "xTc": xTc,
                "xTq": xTq,
                "wqT": wqT,
                "wkT": wkT,
                "wvT": wvT,
                "masks": masks[f],
            }
        )
    return in_maps


def kernel(**inputs: np.ndarray) -> np.ndarray:
    in_maps = build_in_maps(inputs)
    nc = _build_nc()
    res = run_bass_kernel_spmd(nc, in_maps, core_ids=list(range(N_CORES)))

    out = np.empty((B, S, D), dtype=np.float32)
    for c in range(N_CORES):
        b, f = c // 2, c % 2
        o = res.results[c]["out"]  # [1024, 1024] rows in slot order
        for s, qb in enumerate(FOLD_QBLOCKS[f]):
            out[b, qb * QB : (qb + 1) * QB, :] = o[s * QB : (s + 1) * QB, :]
    return out


# revision 5
# speedup vs baseline: 1.0978x; 1.0910x over previous
"""Causal attention (B=4, S=2048, D=1024) on 8 Trainium2 NeuronCores.

v2: pair-split K/V projection with chunked pair-AllGather.

Sharding: 2 cores per batch element; query 256-blocks split by parity
(fold 0: odd blocks, fold 1: even) for causal balance. Context 128-blocks
split by the SAME parity (fold f owns global blocks g%2==f), each core
projects K/V only for its own 1024 context columns; NCH chunked pair
AllGathers exchange them. kt_res / v_res live in AG-output order
([ch, rank, ...]) so the unpack is a few big contiguous DMAs (on the
scalar HW-DGE queue, while the sync queue streams weights/x/masks); the
scores / attn@V loops index through a static global->(ch, r, j) map.
Attention math is identical to the baseline kernel.
"""

import sys

sys.path.insert(0, "/opt/trn_rl_repo")

import ml_dtypes
import numpy as np

import concourse.bass as bass  # noqa: F401
import concourse.mybir as mybir
import concourse.tile as tile
from concourse import bacc
from concourse.bass_utils import run_bass_kernel_spmd

F32 = mybir.dt.float32
BF16 = mybir.dt.bfloat16
AF = mybir.ActivationFunctionType

B, S, D = 4, 2048, 1024
P = 128
DC = D // P  # 8 contraction chunks
OC = D // P  # 8 output-feature chunks
TC = S // P  # 16 context chunks
N_CORES = 8
SLOTS = 4  # query slots of 256 rows per core
QB = 256
KT_COUNTS = [4, 8, 12, 16]
FOLD_QBLOCKS = {0: [1, 3, 5, 7], 1: [0, 2, 4, 6]}
PAIR_DEPTH = [KT_COUNTS[1], KT_COUNTS[3]]  # [8, 16]
N_MASK = sum(PAIR_DEPTH) - 8
SCALE = 1.0 / np.sqrt(np.float32(D))
AG_GROUPS = [[0, 1], [2, 3], [4, 5], [6, 7]]

NCH = 4  # AllGather chunks; each covers 1024/NCH own ctx cols
CW = (S // 2) // NCH  # own ctx cols per chunk
LB = CW // P  # own 128-blocks per chunk
BPC = TC // NCH  # global 128-blocks per chunk (both ranks)
AGW = OC * CW + LB * D  # ag payload cols per chunk (K section + V section)


def _kt_map(kt):
    """Global context 128-block -> (chunk, rank, local j)."""
    ch, rem = divmod(kt, BPC)
    j, r = divmod(rem, 2)[0], rem % 2
    return ch, r, j


def _build_nc(repeat: int = 1):
    nc = bacc.Bacc("TRN2", target_bir_lowering=False, debug=False, num_devices=N_CORES)

    xTc_d = nc.declare_dram_parameter("xTc", [D, S // 2], BF16, isOutput=False)
    xTq_d = nc.declare_dram_parameter("xTq", [D, SLOTS * QB], BF16, isOutput=False)
    wq_d = nc.declare_dram_parameter("wqT", [D, D], BF16, isOutput=False)
    wk_d = nc.declare_dram_parameter("wkT", [D, D], BF16, isOutput=False)
    wv_d = nc.declare_dram_parameter("wvT", [D, D], BF16, isOutput=False)
    mask_d = nc.declare_dram_parameter(
        "masks", [N_MASK, P, 2 * QB], BF16, isOutput=False
    )
    out_d = nc.declare_dram_parameter("out", [SLOTS * QB, D], F32, isOutput=True)

    xTc = xTc_d[:].rearrange("(dc p) t -> p dc t", p=P)  # [128, 8, 1024]
    xTq = xTq_d[:].rearrange("(dc p) q -> p dc q", p=P)  # [128, 8, 1024]
    wq = wq_d[:].rearrange("(dc p) o -> p dc o", p=P)
    wk = wk_d[:].rearrange("(dc p) o -> p dc o", p=P)
    wv = wv_d[:].rearrange("(dc p) o -> p dc o", p=P)
    out_r = out_d[:].rearrange("(qc p) o -> p qc o", p=P)  # [128, 8, 1024]

    with tile.TileContext(nc, pool_alloc_mode="queue") as tc:
      for _rep in range(repeat):
        with tc.tile_pool(name="resident", bufs=1) as res_pool:
            # K^T in AG order: [p, ch, rank, oc, j, c]
            kt_res = res_pool.tile([P, NCH, 2, OC, LB, P], BF16, name="kt_res")
            # V in AG order: [p, ch, rank, j, o]
            v_res = res_pool.tile([P, NCH, 2, LB, D], BF16, name="v_res")
            qt_res = res_pool.tile([P, OC, SLOTS * QB], BF16, name="qt_res")
            ones2 = res_pool.tile([P, 2], BF16, name="ones2")
            nc.vector.memset(ones2[:], 1.0)

            # ---- Phase KV: project own parity ctx, chunked pair-AllGather --
            with (
                tc.tile_pool(name="wk_pool", bufs=1) as wkpool,
                tc.tile_pool(name="wv_pool", bufs=1) as wvpool,
                tc.tile_pool(name="wq_pool", bufs=1) as wqpool,
                tc.tile_pool(name="xq_pool", bufs=1) as xqpool,
                tc.tile_pool(name="xkv_pool", bufs=1) as xpool,
                tc.tile_pool(name="kv_stage", bufs=2) as stpool,
                tc.tile_pool(name="ag_dram", bufs=1, space="DRAM") as dpool,
                tc.tile_pool(name="psum_kv", bufs=4, space="PSUM") as pspool,
            ):
                wk_t = wkpool.tile([P, DC, D], BF16, name="wk_t")
                wv_t = wvpool.tile([P, DC, D], BF16, name="wv_t")
                wq_t = wqpool.tile([P, DC, D], BF16, name="wq_t")
                xq_t = xqpool.tile([P, DC, SLOTS * QB], BF16, name="xq_t")
                x_ts = []
                for dc in range(DC):
                    nc.sync.dma_start(wk_t[:, dc, :], wk[:, dc, :])
                # first two x chunks early so chunk-0/1 compute starts asap
                for ch in range(min(2, NCH)):
                    x_t = xpool.tile([P, DC, CW], BF16, name=f"xkv_t{ch}")
                    for dc in range(DC):
                        nc.sync.dma_start(
                            x_t[:, dc, :], xTc[:, dc, CW * ch : CW * (ch + 1)]
                        )
                    x_ts.append(x_t)
                for dc in range(DC):
                    nc.sync.dma_start(wv_t[:, dc, :], wv[:, dc, :])
                # prefetch Q-phase operands behind the KV-critical loads
                for dc in range(DC):
                    nc.sync.dma_start(wq_t[:, dc, :], wq[:, dc, :])
                    nc.sync.dma_start(xq_t[:, dc, :], xTq[:, dc, :])
                for ch in range(2, NCH):
                    x_t = xpool.tile([P, DC, CW], BF16, name=f"xkv_t{ch}")
                    for dc in range(DC):
                        nc.sync.dma_start(
                            x_t[:, dc, :], xTc[:, dc, CW * ch : CW * (ch + 1)]
                        )
                    x_ts.append(x_t)

                agouts = []
                for ch in range(NCH):
                    x_t = x_ts[ch]
                    kst = stpool.tile([P, OC * CW], BF16, name="k_stage")
                    vst = stpool.tile([P, LB * D], BF16, name="v_stage")
                    # K^T own columns: [o-part, own ctx]
                    for oc in range(OC):
                        ps = pspool.tile([P, CW], F32, name="ps_k", tag="ps_kv")
                        for dc in range(DC):
                            nc.tensor.matmul(
                                ps[:],
                                lhsT=wk_t[:, dc, P * oc : P * (oc + 1)],
                                rhs=x_t[:, dc, :],
                                start=(dc == 0),
                                stop=(dc == DC - 1),
                            )
                        nc.vector.tensor_copy(
                            kst[:, CW * oc : CW * (oc + 1)], ps[:]
                        )
                    # V own rows: [ctx-part, o]
                    for j in range(LB):
                        for ot in range(2):
                            ps = pspool.tile([P, 512], F32, name="ps_v", tag="ps_kv")
                            for dc in range(DC):
                                nc.tensor.matmul(
                                    ps[:],
                                    lhsT=x_t[:, dc, P * j : P * (j + 1)],
                                    rhs=wv_t[:, dc, 512 * ot : 512 * (ot + 1)],
                                    start=(dc == 0),
                                    stop=(dc == DC - 1),
                                )
                            nc.vector.tensor_copy(
                                vst[:, D * j + 512 * ot : D * j + 512 * (ot + 1)],
                                ps[:],
                            )
                    agin = dpool.tile([P, AGW], BF16, name=f"ag_in{ch}")
                    agout = dpool.tile([2, P, AGW], BF16, name=f"ag_out{ch}")
                    nc.sync.dma_start(agin[:, 0 : OC * CW], kst[:])
                    nc.sync.dma_start(agin[:, OC * CW : AGW], vst[:])
                    nc.gpsimd.collective_compute(
                        "AllGather",
                        mybir.AluOpType.bypass,
                        replica_groups=AG_GROUPS,
                        ins=[agin.opt()],
                        outs=[agout.opt()],
                    )
                    agouts.append(agout)

                # ---- Phase Q (overlaps the collectives) --------------------
                with tc.tile_pool(name="psum_q", bufs=4, space="PSUM") as psq:
                    for qt in range(2):
                        for oc in range(OC):
                            ps = psq.tile([P, 512], F32, name="ps_q")
                            for dc in range(DC):
                                nc.tensor.matmul(
                                    ps[:],
                                    lhsT=wq_t[:, dc, P * oc : P * (oc + 1)],
                                    rhs=xq_t[:, dc, 512 * qt : 512 * (qt + 1)],
                                    start=(dc == 0),
                                    stop=(dc == DC - 1),
                                )
                            nc.vector.tensor_copy(
                                qt_res[:, oc, 512 * qt : 512 * (qt + 1)], ps[:]
                            )

                # Unpack AG results (scalar HW-DGE queue; big contiguous DMAs)
                for ch in range(NCH):
                    agout = agouts[ch]
                    agk = agout[:, :, 0 : OC * CW].rearrange("r p c -> p r c")
                    agv = agout[:, :, OC * CW : AGW].rearrange("r p c -> p r c")
                    nc.scalar.dma_start(kt_res[:, ch], agk)
                    nc.scalar.dma_start(v_res[:, ch], agv)

            # ---- Phase A: attention, one slot-pair (512 q) at a time -------
            with (
                tc.tile_pool(name="es_pool", bufs=16) as epool,
                tc.tile_pool(name="mk_pool", bufs=2) as mpool,
                tc.tile_pool(name="ob_pool", bufs=3) as opool,
                tc.tile_pool(name="rc_pool", bufs=2) as rpool,
                tc.tile_pool(name="psum_s", bufs=2, space="PSUM") as pss,
                tc.tile_pool(name="psum_o", bufs=4, space="PSUM") as pso_pool,
                tc.tile_pool(name="psum_d", bufs=2, space="PSUM") as psd_pool,
            ):
                mask_i = 0
                for p in range(2):  # slot pairs (0,1), (2,3)
                    depth = PAIR_DEPTH[p]
                    es_tiles = []
                    for kt in range(depth):
                        kch, kr, kj = _kt_map(kt)
                        ps_s = pss.tile([P, 512], F32, name="ps_s")
                        for oc in range(OC):
                            nc.tensor.matmul(
                                ps_s[:],
                                lhsT=kt_res[:, kch, kr, oc, kj, :],
                                rhs=qt_res[:, oc, 512 * p : 512 * (p + 1)],
                                start=(oc == 0),
                                stop=(oc == OC - 1),
                            )
                        es = epool.tile([P, 512], BF16, name="es")
                        nc.scalar.activation(es[:], ps_s[:], AF.Exp, scale=SCALE)
                        if p == 1 and kt < 8:
                            pass  # both slots fully valid, no mask needed
                        else:
                            mt = mpool.tile([P, 512], BF16, name="mask_t")
                            nc.sync.dma_start(mt[:], mask_d[mask_i])
                            nc.vector.tensor_mul(out=es[:], in0=es[:], in1=mt[:])
                            mask_i += 1
                        es_tiles.append(es)
                    # attn@V: two sweeps (slot A: qcc 0,1; slot B: qcc 2,3)
                    for sw, qccs in enumerate(((0, 1), (2, 3))):
                        sdepth = KT_COUNTS[2 * p + sw]
                        pso = {
                            (qcc, ot): pso_pool.tile([P, 512], F32, name="ps_o")
                            for qcc in qccs
                            for ot in range(2)
                        }
                        psd = {
                            qcc: psd_pool.tile([P, 2], F32, name="ps_d")
                            for qcc in qccs
                        }
                        for kt in range(sdepth):
                            kch, kr, kj = _kt_map(kt)
                            first, last = (kt == 0), (kt == sdepth - 1)
                            for qcc in qccs:
                                lhs = es_tiles[kt][:, P * qcc : P * (qcc + 1)]
                                for ot in range(2):
                                    nc.tensor.matmul(
                                        pso[(qcc, ot)][:],
                                        lhsT=lhs,
                                        rhs=v_res[
                                            :, kch, kr, kj,
                                            512 * ot : 512 * (ot + 1),
                                        ],
                                        start=first,
                                        stop=last,
                                    )
                                nc.tensor.matmul(
                                    psd[qcc][:],
                                    lhsT=lhs,
                                    rhs=ones2[:],
                                    start=first,
                                    stop=last,
                                )
                        for qcc in qccs:
                            rc = rpool.tile([P, 1], F32, name="rc")
                            nc.vector.reciprocal(rc[:], psd[qcc][:, 0:1])
                            for ot in range(2):
                                ob = opool.tile([P, 512], F32, name="ob")
                                nc.scalar.activation(
                                    ob[:], pso[(qcc, ot)][:], AF.Copy, scale=rc[:]
                                )
                                nc.sync.dma_start(
                                    out_r[:, 4 * p + qcc, 512 * ot : 512 * (ot + 1)],
                                    ob[:],
                                )

    nc.compile()
    if not nc.is_finalized():
        nc.finalize()
    return nc


def _build_masks(fold: int) -> np.ndarray:
    """0/1 masks [N_MASK, 128, 512]; cols 0:256 = slot 2p, 256:512 = slot 2p+1."""
    tiles = []
    ki = np.arange(P)[:, None]
    qi = np.arange(QB)[None, :]
    for p in range(2):
        lo = 8 if p == 1 else 0  # pair1 kt<8 is fully valid for both folds
        for kt in range(lo, PAIR_DEPTH[p]):
            k0 = kt * P
            halves = []
            for s in (2 * p, 2 * p + 1):
                q0 = FOLD_QBLOCKS[fold][s] * QB
                halves.append(((q0 + qi) >= (k0 + ki)).astype(np.float32))
            tiles.append(np.concatenate(halves, axis=1))
    return np.ascontiguousarray(np.stack(tiles).astype(ml_dtypes.bfloat16))


def build_in_maps(inputs):
    x = np.asarray(inputs["inputs"], dtype=np.float32)
    bf = ml_dtypes.bfloat16
    wqT = np.ascontiguousarray(np.asarray(inputs["Wq"], dtype=np.float32).T.astype(bf))
    wkT = np.ascontiguousarray(np.asarray(inputs["Wk"], dtype=np.float32).T.astype(bf))
    wvT = np.ascontiguousarray(np.asarray(inputs["Wv"], dtype=np.float32).T.astype(bf))

    masks = {f: _build_masks(f) for f in (0, 1)}
    in_maps = []
    for c in range(N_CORES):
        b, f = c // 2, c % 2
        xT = np.ascontiguousarray(x[b].T.astype(bf))  # [D, S]
        xTq = np.ascontiguousarray(
            np.concatenate(
                [xT[:, qb * QB : (qb + 1) * QB] for qb in FOLD_QBLOCKS[f]], axis=1
            )
        )
        # Own parity context blocks, packed: local l -> global block 2*l + f.
        xTc = np.ascontiguousarray(
            np.concatenate(
                [xT[:, g * P : (g + 1) * P] for g in range(f, TC, 2)], axis=1
            )
        )
        in_maps.append(
            {
                "xTc": xTc,
                "xTq": xTq,
                "wqT": wqT,
                "wkT": wkT,
                "wvT": wvT,
                "masks": masks[f],
            }
        )
    return in_maps


def kernel(**inputs: np.ndarray) -> np.ndarray:
    in_maps = build_in_maps(inputs)
    nc = _build_nc()
    res = run_bass_kernel_spmd(nc, in_maps, core_ids=list(range(N_CORES)))

    out = np.empty((B, S, D), dtype=np.float32)
    for c in range(N_CORES):
        b, f = c // 2, c % 2
        o = res.results[c]["out"]  # [1024, 1024] rows in slot order
        for s, qb in enumerate(FOLD_QBLOCKS[f]):
            out[b, qb * QB : (qb + 1) * QB, :] = o[s * QB : (s + 1) * QB, :]
    return out


# revision 6
# speedup vs baseline: 1.0981x; 1.0003x over previous
"""Causal attention (B=4, S=2048, D=1024) on 8 Trainium2 NeuronCores.

v2: pair-split K/V projection with chunked pair-AllGather.

Sharding: 2 cores per batch element; query 256-blocks split by parity
(fold 0: odd blocks, fold 1: even) for causal balance. Context 128-blocks
split by the SAME parity (fold f owns global blocks g%2==f), each core
projects K/V only for its own 1024 context columns; NCH chunked pair
AllGathers exchange them. kt_res / v_res live in AG-output order
([ch, rank, ...]) so the unpack is a few big contiguous DMAs (on the
scalar HW-DGE queue, while the sync queue streams weights/x/masks); the
scores / attn@V loops index through a static global->(ch, r, j) map.
Attention math is identical to the baseline kernel.
"""

import sys

sys.path.insert(0, "/opt/trn_rl_repo")

import ml_dtypes
import numpy as np

import concourse.bass as bass  # noqa: F401
import concourse.mybir as mybir
import concourse.tile as tile
from concourse import bacc
from concourse.bass_utils import run_bass_kernel_spmd

F32 = mybir.dt.float32
BF16 = mybir.dt.bfloat16
AF = mybir.ActivationFunctionType

B, S, D = 4, 2048, 1024
P = 128
DC = D // P  # 8 contraction chunks
OC = D // P  # 8 output-feature chunks
TC = S // P  # 16 context chunks
N_CORES = 8
SLOTS = 4  # query slots of 256 rows per core
QB = 256
KT_COUNTS = [4, 8, 12, 16]
FOLD_QBLOCKS = {0: [1, 3, 5, 7], 1: [0, 2, 4, 6]}
PAIR_DEPTH = [KT_COUNTS[1], KT_COUNTS[3]]  # [8, 16]
N_MASK = sum(PAIR_DEPTH) - 8
SCALE = 1.0 / np.sqrt(np.float32(D))
AG_GROUPS = [[0, 1], [2, 3], [4, 5], [6, 7]]

NCH = 4  # AllGather chunks; each covers 1024/NCH own ctx cols
CW = (S // 2) // NCH  # own ctx cols per chunk
LB = CW // P  # own 128-blocks per chunk
BPC = TC // NCH  # global 128-blocks per chunk (both ranks)
AGW = OC * CW + LB * D  # ag payload cols per chunk (K section + V section)


def _kt_map(kt):
    """Global context 128-block -> (chunk, rank, local j)."""
    ch, rem = divmod(kt, BPC)
    j, r = divmod(rem, 2)[0], rem % 2
    return ch, r, j


def _build_nc(repeat: int = 1):
    nc = bacc.Bacc("TRN2", target_bir_lowering=False, debug=False, num_devices=N_CORES)

    xTc_d = nc.declare_dram_parameter("xTc", [D, S // 2], BF16, isOutput=False)
    xTq_d = nc.declare_dram_parameter("xTq", [D, SLOTS * QB], BF16, isOutput=False)
    wq_d = nc.declare_dram_parameter("wqT", [D, D], BF16, isOutput=False)
    wk_d = nc.declare_dram_parameter("wkT", [D, D], BF16, isOutput=False)
    wv_d = nc.declare_dram_parameter("wvT", [D, D], BF16, isOutput=False)
    mask_d = nc.declare_dram_parameter(
        "masks", [N_MASK, P, 2 * QB], BF16, isOutput=False
    )
    out_d = nc.declare_dram_parameter("out", [SLOTS * QB, D], F32, isOutput=True)

    xTc = xTc_d[:].rearrange("(dc p) t -> p dc t", p=P)  # [128, 8, 1024]
    xTq = xTq_d[:].rearrange("(dc p) q -> p dc q", p=P)  # [128, 8, 1024]
    wq = wq_d[:].rearrange("(dc p) o -> p dc o", p=P)
    wk = wk_d[:].rearrange("(dc p) o -> p dc o", p=P)
    wv = wv_d[:].rearrange("(dc p) o -> p dc o", p=P)
    out_r = out_d[:].rearrange("(qc p) o -> p qc o", p=P)  # [128, 8, 1024]

    with tile.TileContext(nc, pool_alloc_mode="queue") as tc:
      for _rep in range(repeat):
        with tc.tile_pool(name="resident", bufs=1) as res_pool:
            # K^T in AG order: [p, ch, rank, oc, j, c]
            kt_res = res_pool.tile([P, NCH, 2, OC, LB, P], BF16, name="kt_res")
            # V in AG order: [p, ch, rank, j, o]
            v_res = res_pool.tile([P, NCH, 2, LB, D], BF16, name="v_res")
            qt_res = res_pool.tile([P, OC, SLOTS * QB], BF16, name="qt_res")
            ones2 = res_pool.tile([P, 2], BF16, name="ones2")
            nc.vector.memset(ones2[:], 1.0)

            # ---- Phase KV: project own parity ctx, chunked pair-AllGather --
            with (
                tc.tile_pool(name="wk_pool", bufs=1) as wkpool,
                tc.tile_pool(name="wv_pool", bufs=1) as wvpool,
                tc.tile_pool(name="wq_pool", bufs=1) as wqpool,
                tc.tile_pool(name="xq_pool", bufs=1) as xqpool,
                tc.tile_pool(name="xkv_pool", bufs=1) as xpool,
                tc.tile_pool(name="kv_stage", bufs=2) as stpool,
                tc.tile_pool(name="ag_dram", bufs=1, space="DRAM") as dpool,
                tc.tile_pool(name="psum_kv", bufs=4, space="PSUM") as pspool,
            ):
                wk_t = wkpool.tile([P, DC, D], BF16, name="wk_t")
                wv_t = wvpool.tile([P, DC, D], BF16, name="wv_t")
                wq_t = wqpool.tile([P, DC, D], BF16, name="wq_t")
                xq_t = xqpool.tile([P, DC, SLOTS * QB], BF16, name="xq_t")
                x_ts = []
                for dc in range(DC):
                    nc.sync.dma_start(wk_t[:, dc, :], wk[:, dc, :])
                # first two x chunks early so chunk-0/1 compute starts asap
                for ch in range(min(2, NCH)):
                    x_t = xpool.tile([P, DC, CW], BF16, name=f"xkv_t{ch}")
                    for dc in range(DC):
                        nc.sync.dma_start(
                            x_t[:, dc, :], xTc[:, dc, CW * ch : CW * (ch + 1)]
                        )
                    x_ts.append(x_t)
                for dc in range(DC):
                    nc.sync.dma_start(wv_t[:, dc, :], wv[:, dc, :])
                # Q-phase operands + late x chunks prefetch on the scalar
                # HW-DGE queue so chunk exports aren't stuck behind them on
                # the sync queue.
                for dc in range(DC):
                    nc.scalar.dma_start(wq_t[:, dc, :], wq[:, dc, :])
                    nc.scalar.dma_start(xq_t[:, dc, :], xTq[:, dc, :])
                for ch in range(2, NCH):
                    x_t = xpool.tile([P, DC, CW], BF16, name=f"xkv_t{ch}")
                    for dc in range(DC):
                        nc.scalar.dma_start(
                            x_t[:, dc, :], xTc[:, dc, CW * ch : CW * (ch + 1)]
                        )
                    x_ts.append(x_t)

                agouts = []
                for ch in range(NCH):
                    x_t = x_ts[ch]
                    kst = stpool.tile([P, OC * CW], BF16, name="k_stage")
                    vst = stpool.tile([P, LB * D], BF16, name="v_stage")
                    # K^T own columns: [o-part, own ctx]
                    for oc in range(OC):
                        ps = pspool.tile([P, CW], F32, name="ps_k", tag="ps_kv")
                        for dc in range(DC):
                            nc.tensor.matmul(
                                ps[:],
                                lhsT=wk_t[:, dc, P * oc : P * (oc + 1)],
                                rhs=x_t[:, dc, :],
                                start=(dc == 0),
                                stop=(dc == DC - 1),
                            )
                        nc.vector.tensor_copy(
                            kst[:, CW * oc : CW * (oc + 1)], ps[:]
                        )
                    # V own rows: [ctx-part, o]
                    for j in range(LB):
                        for ot in range(2):
                            ps = pspool.tile([P, 512], F32, name="ps_v", tag="ps_kv")
                            for dc in range(DC):
                                nc.tensor.matmul(
                                    ps[:],
                                    lhsT=x_t[:, dc, P * j : P * (j + 1)],
                                    rhs=wv_t[:, dc, 512 * ot : 512 * (ot + 1)],
                                    start=(dc == 0),
                                    stop=(dc == DC - 1),
                                )
                            nc.vector.tensor_copy(
                                vst[:, D * j + 512 * ot : D * j + 512 * (ot + 1)],
                                ps[:],
                            )
                    agin = dpool.tile([P, AGW], BF16, name=f"ag_in{ch}")
                    agout = dpool.tile([2, P, AGW], BF16, name=f"ag_out{ch}")
                    nc.sync.dma_start(agin[:, 0 : OC * CW], kst[:])
                    nc.sync.dma_start(agin[:, OC * CW : AGW], vst[:])
                    nc.gpsimd.collective_compute(
                        "AllGather",
                        mybir.AluOpType.bypass,
                        replica_groups=AG_GROUPS,
                        ins=[agin.opt()],
                        outs=[agout.opt()],
                    )
                    agouts.append(agout)

                # ---- Phase Q (overlaps the collectives) --------------------
                with tc.tile_pool(name="psum_q", bufs=4, space="PSUM") as psq:
                    for qt in range(2):
                        for oc in range(OC):
                            ps = psq.tile([P, 512], F32, name="ps_q")
                            for dc in range(DC):
                                nc.tensor.matmul(
                                    ps[:],
                                    lhsT=wq_t[:, dc, P * oc : P * (oc + 1)],
                                    rhs=xq_t[:, dc, 512 * qt : 512 * (qt + 1)],
                                    start=(dc == 0),
                                    stop=(dc == DC - 1),
                                )
                            nc.vector.tensor_copy(
                                qt_res[:, oc, 512 * qt : 512 * (qt + 1)], ps[:]
                            )

                # Unpack AG results (scalar HW-DGE queue; big contiguous DMAs)
                for ch in range(NCH):
                    agout = agouts[ch]
                    agk = agout[:, :, 0 : OC * CW].rearrange("r p c -> p r c")
                    agv = agout[:, :, OC * CW : AGW].rearrange("r p c -> p r c")
                    nc.scalar.dma_start(kt_res[:, ch], agk)
                    nc.scalar.dma_start(v_res[:, ch], agv)

            # ---- Phase A: attention, one slot-pair (512 q) at a time -------
            with (
                tc.tile_pool(name="es_pool", bufs=16) as epool,
                tc.tile_pool(name="mk_pool", bufs=2) as mpool,
                tc.tile_pool(name="ob_pool", bufs=3) as opool,
                tc.tile_pool(name="rc_pool", bufs=2) as rpool,
                tc.tile_pool(name="psum_s", bufs=2, space="PSUM") as pss,
                tc.tile_pool(name="psum_o", bufs=4, space="PSUM") as pso_pool,
                tc.tile_pool(name="psum_d", bufs=2, space="PSUM") as psd_pool,
            ):
                mask_i = 0
                for p in range(2):  # slot pairs (0,1), (2,3)
                    depth = PAIR_DEPTH[p]
                    es_tiles = []
                    for kt in range(depth):
                        kch, kr, kj = _kt_map(kt)
                        ps_s = pss.tile([P, 512], F32, name="ps_s")
                        for oc in range(OC):
                            nc.tensor.matmul(
                                ps_s[:],
                                lhsT=kt_res[:, kch, kr, oc, kj, :],
                                rhs=qt_res[:, oc, 512 * p : 512 * (p + 1)],
                                start=(oc == 0),
                                stop=(oc == OC - 1),
                            )
                        es = epool.tile([P, 512], BF16, name="es")
                        nc.scalar.activation(es[:], ps_s[:], AF.Exp, scale=SCALE)
                        if p == 1 and kt < 8:
                            pass  # both slots fully valid, no mask needed
                        else:
                            mt = mpool.tile([P, 512], BF16, name="mask_t")
                            nc.sync.dma_start(mt[:], mask_d[mask_i])
                            nc.vector.tensor_mul(out=es[:], in0=es[:], in1=mt[:])
                            mask_i += 1
                        es_tiles.append(es)
                    # attn@V: two sweeps (slot A: qcc 0,1; slot B: qcc 2,3)
                    for sw, qccs in enumerate(((0, 1), (2, 3))):
                        sdepth = KT_COUNTS[2 * p + sw]
                        pso = {
                            (qcc, ot): pso_pool.tile([P, 512], F32, name="ps_o")
                            for qcc in qccs
                            for ot in range(2)
                        }
                        psd = {
                            qcc: psd_pool.tile([P, 2], F32, name="ps_d")
                            for qcc in qccs
                        }
                        for kt in range(sdepth):
                            kch, kr, kj = _kt_map(kt)
                            first, last = (kt == 0), (kt == sdepth - 1)
                            for qcc in qccs:
                                lhs = es_tiles[kt][:, P * qcc : P * (qcc + 1)]
                                for ot in range(2):
                                    nc.tensor.matmul(
                                        pso[(qcc, ot)][:],
                                        lhsT=lhs,
                                        rhs=v_res[
                                            :, kch, kr, kj,
                                            512 * ot : 512 * (ot + 1),
                                        ],
                                        start=first,
                                        stop=last,
                                    )
                                nc.tensor.matmul(
                                    psd[qcc][:],
                                    lhsT=lhs,
                                    rhs=ones2[:],
                                    start=first,
                                    stop=last,
                                )
                        for qcc in qccs:
                            rc = rpool.tile([P, 1], F32, name="rc")
                            nc.vector.reciprocal(rc[:], psd[qcc][:, 0:1])
                            for ot in range(2):
                                ob = opool.tile([P, 512], F32, name="ob")
                                nc.scalar.activation(
                                    ob[:], pso[(qcc, ot)][:], AF.Copy, scale=rc[:]
                                )
                                nc.sync.dma_start(
                                    out_r[:, 4 * p + qcc, 512 * ot : 512 * (ot + 1)],
                                    ob[:],
                                )

    nc.compile()
    if not nc.is_finalized():
        nc.finalize()
    return nc


def _build_masks(fold: int) -> np.ndarray:
    """0/1 masks [N_MASK, 128, 512]; cols 0:256 = slot 2p, 256:512 = slot 2p+1."""
    tiles = []
    ki = np.arange(P)[:, None]
    qi = np.arange(QB)[None, :]
    for p in range(2):
        lo = 8 if p == 1 else 0  # pair1 kt<8 is fully valid for both folds
        for kt in range(lo, PAIR_DEPTH[p]):
            k0 = kt * P
            halves = []
            for s in (2 * p, 2 * p + 1):
                q0 = FOLD_QBLOCKS[fold][s] * QB
                halves.append(((q0 + qi) >= (k0 + ki)).astype(np.float32))
            tiles.append(np.concatenate(halves, axis=1))
    return np.ascontiguousarray(np.stack(tiles).astype(ml_dtypes.bfloat16))


def build_in_maps(inputs):
    x = np.asarray(inputs["inputs"], dtype=np.float32)
    bf = ml_dtypes.bfloat16
    wqT = np.ascontiguousarray(np.asarray(inputs["Wq"], dtype=np.float32).T.astype(bf))
    wkT = np.ascontiguousarray(np.asarray(inputs["Wk"], dtype=np.float32).T.astype(bf))
    wvT = np.ascontiguousarray(np.asarray(inputs["Wv"], dtype=np.float32).T.astype(bf))

    masks = {f: _build_masks(f) for f in (0, 1)}
    in_maps = []
    for c in range(N_CORES):
        b, f = c // 2, c % 2
        xT = np.ascontiguousarray(x[b].T.astype(bf))  # [D, S]
        xTq = np.ascontiguousarray(
            np.concatenate(
                [xT[:, qb * QB : (qb + 1) * QB] for qb in FOLD_QBLOCKS[f]], axis=1
            )
        )
        # Own parity context blocks, packed: local l -> global block 2*l + f.
        xTc = np.ascontiguousarray(
            np.concatenate(
                [xT[:, g * P : (g + 1) * P] for g in range(f, TC, 2)], axis=1
            )
        )
        in_maps.append(
            {
                "xTc": xTc,
                "xTq": xTq,
                "wqT": wqT,
                "wkT": wkT,
                "wvT": wvT,
                "masks": masks[f],
            }
        )
    return in_maps


def kernel(**inputs: np.ndarray) -> np.ndarray:
    in_maps = build_in_maps(inputs)
    nc = _build_nc()
    res = run_bass_kernel_spmd(nc, in_maps, core_ids=list(range(N_CORES)))

    out = np.empty((B, S, D), dtype=np.float32)
    for c in range(N_CORES):
        b, f = c // 2, c % 2
        o = res.results[c]["out"]  # [1024, 1024] rows in slot order
        for s, qb in enumerate(FOLD_QBLOCKS[f]):
            out[b, qb * QB : (qb + 1) * QB, :] = o[s * QB : (s + 1) * QB, :]
    return out
